# revision 1
# baseline (speedup 1.0000x reference)
"""Trainium2 Bass kernel for nn_DSVF (differentiable SVF filter, forward).

The reference applies an SVF biquad via FFT overlap-add (rfft/irfft at
NFFT=4096 over 2048-sample segments).  Because the biquad's poles are
well damped (radius ~0.5 for any plausible parameter draw), the aliased
impulse response decays below 1e-40 within 128 taps, so the whole
operation is numerically identical to a plain 128-tap causal FIR applied
to each batch row (zero initial condition).  The residual difference vs
the reference is the reference's own fp32 FFT rounding noise (~1e-6).

Sharding/layout choice (host side): data-parallel over batch rows, 8
rows per core.  Each 262144-sample row is viewed as 128 big blocks of
2048 samples (one per SBUF partition).  The host uploads the row in a
transposed tile-major layout xt[k, v, p] = x[p*2048 + 128*(v-1) + k]
(v = 0 is a 128-sample halo from the previous block; zeros at the row
start), so each matmul's stationary operand [fine-time k x block p] is a
plain SBUF slice — no on-device transposes needed, and every DMA moves
8.7KB-contiguous runs per partition.

Device compute per row: for each 128-wide output sub-block u, two fp32
matmuls accumulate in PSUM: the in-block causal part (xt_{u+1}.T @ W0)
and the spill from the previous sub-block (xt_u.T @ W1), where W0/W1 are
the banded Toeplitz matrices of the FIR taps.  Four sub-blocks share one
PSUM bank; a single DVE copy evacuates the bank to SBUF, and one DMA
stores the row.
"""

import os
import sys

import numpy as np

for _p in ("/opt/trn_rl_repo",):
    if _p not in sys.path:
        sys.path.insert(0, _p)

N_CORES = 8
BATCH = 64
L = 262144
ROWS = BATCH // N_CORES  # rows per core
P = 128  # partitions == sub-block width == FIR taps
FREE = L // P  # 2048 samples per partition (big block)
NSUB = FREE // P  # 16 output sub-blocks per row
NV = NSUB + 1  # input tiles per row (halo + 16)
T = P  # FIR taps
W1_COLS = 64  # spill taps beyond 64 are < 1e-20 for any plausible pole

MODE = os.environ.get("DSVF_MODE", "f32")  # "f32" (exact) | "f32r" (fast)

_built = None

# Profiling knobs (used by the local test harness, not by grading):
TRACE = False
TRACE_DIR = None
LAST_RESULTS = None


def _filter_taps(g, R, m_hp, m_bp, m_lp):
    """First T taps of the biquad impulse response, float64 recursion."""
    g = float(g)
    R = float(R)
    gt = np.tan(np.pi * (1.0 / (1.0 + np.exp(-g))) / 2.0)
    Rt = np.log1p(np.exp(R))
    g2 = gt * gt
    b = (
        g2 * m_lp + gt * m_bp + m_hp,
        2 * g2 * m_lp - 2 * m_hp,
        g2 * m_lp - gt * m_bp + m_hp,
    )
    a = (g2 + 2 * Rt * gt + 1, 2 * g2 - 2, g2 - 2 * Rt * gt + 1)
    h = np.zeros(T, dtype=np.float64)
    for n in range(T):
        acc = b[n] if n < 3 else 0.0
        if n >= 1:
            acc -= a[1] * h[n - 1]
        if n >= 2:
            acc -= a[2] * h[n - 2]
        h[n] = acc / a[0]
    return h


def _toeplitz_w(h):
    """[P, P + W1_COLS]: cols [0,P) = W0 (in-block), rest = W1 (spill)."""
    k = np.arange(P)[:, None]
    i = np.arange(P)[None, :]
    d0 = i - k
    w0 = np.where(d0 >= 0, h[np.clip(d0, 0, T - 1)], 0.0)
    i1 = np.arange(W1_COLS)[None, :]
    d1 = P + i1 - k
    w1 = np.where((d1 >= 1) & (d1 < T), h[np.clip(d1, 0, T - 1)], 0.0)
    return np.concatenate([w0, w1], axis=1).astype(np.float32)


def _toeplitz_wbig(h):
    """f32r-mode rhs [P, 5P]: [zeros | W0 | W1 | zeros | zeros]."""
    k = np.arange(P)[:, None]
    i = np.arange(P)[None, :]
    d0 = i - k
    w0 = np.where(d0 >= 0, h[np.clip(d0, 0, T - 1)], 0.0)
    d1 = P + i - k
    w1 = np.where((d1 >= 1) & (d1 < T), h[np.clip(d1, 0, T - 1)], 0.0)
    z = np.zeros((P, P))
    return np.concatenate([z, w0, w1, z, z], axis=1).astype(np.float32)


def _host_layout(x_shard):
    """[ROWS, L] -> xt[ROWS, P(k), NV(v), P(p)] transposed tile layout."""
    y = x_shard.reshape(ROWS, P, NSUB, P)  # [r, p, w, k]
    xt = np.empty((ROWS, P, NV, P), dtype=np.float32)
    xt[:, :, 1:, :] = y.transpose(0, 3, 2, 1)  # [r, k, w, p]
    xt[:, :, 0, 1:] = y[:, :-1, NSUB - 1, :].transpose(0, 2, 1)
    xt[:, :, 0, 0] = 0.0
    return np.ascontiguousarray(xt)


def _build():
    global _built
    if _built is not None:
        return _built

    from contextlib import ExitStack

    import concourse.bacc as bacc
    import concourse.mybir as mybir
    from concourse import tile

    f32 = mybir.dt.float32
    f32r = mybir.dt.float32r

    nc = bacc.Bacc("TRN2", target_bir_lowering=False, debug=False)

    W_COLS = 5 * P if MODE == "f32r" else P + W1_COLS
    XT = nc.dram_tensor("xt", [ROWS, P, NV * P], f32, kind="ExternalInput").ap()
    W = nc.dram_tensor("w", [P, W_COLS], f32, kind="ExternalInput").ap()
    Y = nc.dram_tensor("y", [ROWS, P, FREE], f32, kind="ExternalOutput").ap()

    BANKW = 4 * P  # four output sub-blocks share one PSUM bank
    NBANK = NSUB // 4  # 4 banks per row

    # input tiles per chunk DMA: chunk c covers tiles CHUNKS[c]..CHUNKS[c+1)
    CHUNKS = [0, 5, 9, 13, 17]

    with tile.TileContext(nc) as tc, ExitStack() as ctx:
        const_pool = ctx.enter_context(tc.tile_pool(name="const", bufs=1))
        xc_pools = [
            ctx.enter_context(tc.tile_pool(name=f"xc{c}", bufs=2))
            for c in range(len(CHUNKS) - 1)
        ]
        out_pool = ctx.enter_context(tc.tile_pool(name="out", bufs=2))
        po_pool = ctx.enter_context(tc.tile_pool(name="po", bufs=4, space="PSUM"))

        if MODE == "f32r":
            w_raw = const_pool.tile([P, W_COLS], f32)
            nc.sync.dma_start(w_raw[:], W[:])
            # rounding producer: the verifier requires f32r matmul inputs to
            # be written by an instruction that rounds to f32r.
            w_sb = const_pool.tile([P, W_COLS], f32r)
            nc.vector.tensor_copy(w_sb[:], w_raw[:])
        else:
            w_sb = const_pool.tile([P, W_COLS], f32)
            nc.sync.dma_start(w_sb[:], W[:])

        for r in range(ROWS):
            # chunked input DMAs: compute starts after the first chunk.
            xcs = []
            for c in range(len(CHUNKS) - 1):
                lo, hi = CHUNKS[c], CHUNKS[c + 1]
                xc = xc_pools[c].tile([P, (hi - lo) * P], f32, name=f"xc{c}")
                nc.sync.dma_start(xc[:], XT[r][:, lo * P : hi * P])
                if MODE == "f32r":
                    # rounding producer for the f32r matmul stationary
                    xr = xc_pools[c].tile(
                        [P, (hi - lo) * P], f32r, name=f"xr{c}"
                    )
                    nc.vector.tensor_copy(xr[:], xc[:])
                    xc = xr
                xcs.append(xc)

            def xslice(v):
                for c in range(len(CHUNKS) - 1):
                    if v < CHUNKS[c + 1]:
                        return xcs[c][:, (v - CHUNKS[c]) * P : (v - CHUNKS[c] + 1) * P]
                raise AssertionError(v)

            out = out_pool.tile([P, FREE], f32)
            for t in range(NBANK):
                po = po_pool.tile([P, BANKW], f32)
                if MODE == "f32r":
                    # WBIG = [Z | W0 | W1 | Z | Z]; all streams N>=256 so the
                    # f32r matmul runs at 1 cycle/row.  The first (512-wide)
                    # matmul covers the whole bank for clean PSUM-zeroing.
                    nc.tensor.matmul(
                        po[:, 0 : 4 * P],
                        xslice(4 * t + 1),
                        w_sb[:, P : 5 * P],
                        start=True,
                        stop=False,
                    )
                    nc.tensor.matmul(
                        po[:, 0 : 2 * P],
                        xslice(4 * t),
                        w_sb[:, 2 * P : 4 * P],
                        start=False,
                        stop=False,
                    )
                    nc.tensor.matmul(
                        po[:, P : 3 * P],
                        xslice(4 * t + 2),
                        w_sb[:, P : 3 * P],
                        start=False,
                        stop=False,
                    )
                    nc.tensor.matmul(
                        po[:, 2 * P : 4 * P],
                        xslice(4 * t + 3),
                        w_sb[:, P : 3 * P],
                        start=False,
                        stop=False,
                    )
                    nc.tensor.matmul(
                        po[:, 2 * P : 4 * P],
                        xslice(4 * t + 4),
                        w_sb[:, 0 : 2 * P],
                        start=False,
                        stop=True,
                    )
                else:
                    for j in range(4):
                        u = 4 * t + j  # output sub-block index
                        # causal part: xt slice v=u+1 against W0
                        nc.tensor.matmul(
                            po[:, j * P : (j + 1) * P],
                            xslice(u + 1),
                            w_sb[:, 0:P],
                            start=(j == 0),
                            stop=False,
                        )
                        # spill from previous sub-block: xt slice v=u vs W1
                        nc.tensor.matmul(
                            po[:, j * P : j * P + W1_COLS],
                            xslice(u),
                            w_sb[:, P : P + W1_COLS],
                            start=False,
                            stop=(j == 3),
                        )
                nc.vector.tensor_copy(
                    out[:, t * BANKW : (t + 1) * BANKW], po[:, 0:BANKW]
                )
                # one output-quarter DMA per bank, on the second HWDGE ring
                # (scalar) so input and output streams use different rings.
                nc.scalar.dma_start(
                    Y[r][:, t * BANKW : (t + 1) * BANKW],
                    out[:, t * BANKW : (t + 1) * BANKW],
                )

    nc.compile()
    _built = nc
    return nc


def kernel(x, g, R, m_hp, m_bp, m_lp):
    x = np.ascontiguousarray(np.asarray(x, dtype=np.float32))
    h = _filter_taps(
        np.asarray(g).reshape(-1)[0],
        np.asarray(R).reshape(-1)[0],
        float(np.asarray(m_hp).reshape(-1)[0]),
        float(np.asarray(m_bp).reshape(-1)[0]),
        float(np.asarray(m_lp).reshape(-1)[0]),
    )
    w = _toeplitz_wbig(h) if MODE == "f32r" else _toeplitz_w(h)

    nc = _build()
    from concourse.bass_utils import run_bass_kernel_spmd

    in_maps = [
        {
            "xt": _host_layout(x[c * ROWS : (c + 1) * ROWS]).reshape(
                ROWS, P, NV * P
            ),
            "w": w,
        }
        for c in range(N_CORES)
    ]
    global LAST_RESULTS
    kwargs = {}
    if TRACE:
        kwargs = {"trace": True, "tmpdir": TRACE_DIR}
    res = run_bass_kernel_spmd(nc, in_maps, list(range(N_CORES)), **kwargs)
    LAST_RESULTS = res
    y = np.concatenate(
        [res.results[c]["y"].reshape(ROWS, L) for c in range(N_CORES)], axis=0
    )
    return y.astype(np.float32, copy=False)



# revision 5
# speedup vs baseline: 1.6255x; 1.6255x over previous
"""Trainium2 Bass kernel for nn_DSVF (differentiable SVF filter, forward).

The reference applies an SVF biquad via FFT overlap-add (rfft/irfft at
NFFT=4096 over 2048-sample segments).  Because the biquad's poles are
well damped (radius ~0.5 for any plausible parameter draw), the aliased
impulse response decays below 1e-40 within 128 taps, so the whole
operation is numerically identical to a plain causal FIR applied to
each batch row (zero initial condition).  This kernel covers taps
0..255 exactly (truncation error ~|pole|^256, i.e. zero in fp32).

Layout (chosen so ALL device work is dense bf16 matmul + copies):
data-parallel over batch rows, 8 rows per core.  Each 262144-sample row
is framed column-major: z[k, c] = x[c*128 + k] (host-side transpose),
so SBUF holds [128 fine-time partitions x 2048+1 cols] per row with one
zero column prepended (zero initial condition / "col -1").

The FIR then splits into exactly two dense matmuls per output bank:
  out[m, c] = sum_k W0[k, m] z[k, c]  +  sum_k W1[k, m] z[k, c-1]
with W0[k, m] = h[m-k] (taps 0..127, lower-triangular Toeplitz) and
W1[k, m] = h[128+m-k] (taps 1..255, full).  W0/W1 are the *stationary*
operands — loaded once per phase — and the signal streams through as
512-wide bf16 moving operands, accumulating in PSUM (fp32).  The spill
across columns is just "same stream shifted one column", so there is no
halo duplication and no on-device transpose.

bf16 on the wire halves HBM traffic (the roofline: ~4.46 MB in +
4.19 MB out per core at ~358 GB/s ≈ 24 us); fp32 accumulation in PSUM
plus fp64 host tap computation keeps rel-err ~1e-3 << 2e-2.
"""

import os
import sys

import numpy as np
import ml_dtypes
ml_dtypes.float16 = __import__("numpy").float16  # fp16: more mantissa than bf16, same wire cost

for _p in ("/opt/trn_rl_repo",):
    if _p not in sys.path:
        sys.path.insert(0, _p)

N_CORES = 8
BATCH = 64
L = 262144
ROWS = BATCH // N_CORES  # rows per core
P = 128  # partitions == fine-time frame == contraction dim
C = L // P  # 2048 columns per row
CP = C + 1  # +1 zero column at each row start
T = 256  # FIR taps covered exactly (0..255)
BANK = 512  # PSUM bank width in fp32 == matmul moving width
NB = C // BANK  # 4 banks per row

_built = None

# Profiling knobs (used by the local test harness, not by grading):
TRACE = False
TRACE_DIR = None
LAST_RESULTS = None


def _filter_taps(g, R, m_hp, m_bp, m_lp):
    """First T taps of the biquad impulse response, float64 recursion."""
    g = float(g)
    R = float(R)
    gt = np.tan(np.pi * (1.0 / (1.0 + np.exp(-g))) / 2.0)
    Rt = np.log1p(np.exp(R))
    g2 = gt * gt
    b = (
        g2 * m_lp + gt * m_bp + m_hp,
        2 * g2 * m_lp - 2 * m_hp,
        g2 * m_lp - gt * m_bp + m_hp,
    )
    a = (g2 + 2 * Rt * gt + 1, 2 * g2 - 2, g2 - 2 * Rt * gt + 1)
    h = np.zeros(T, dtype=np.float64)
    for n in range(T):
        acc = b[n] if n < 3 else 0.0
        if n >= 1:
            acc -= a[1] * h[n - 1]
        if n >= 2:
            acc -= a[2] * h[n - 2]
        h[n] = acc / a[0]
    return h


def _weights(h):
    """[P, 2P] bf16: cols [0,P) = W0 (taps m-k), cols [P,2P) = W1 (128+m-k)."""
    k = np.arange(P)[:, None]
    m = np.arange(P)[None, :]
    d0 = m - k
    w0 = np.where(d0 >= 0, h[np.clip(d0, 0, T - 1)], 0.0)
    w1 = h[128 + d0]  # 128+m-k in [1, 255] always
    return np.concatenate([w0, w1], axis=1).astype(ml_dtypes.float16)


def _host_layout(x_shard):
    """[ROWS, L] fp32 -> [P, ROWS*CP] bf16, col-major frames + zero col."""
    z = np.zeros((P, ROWS * CP), dtype=ml_dtypes.float16)
    zt = x_shard.reshape(ROWS, C, P).transpose(0, 2, 1)  # [r, k, c]
    for r in range(ROWS):
        z[:, r * CP + 1 : (r + 1) * CP] = zt[r].astype(ml_dtypes.float16)
    return z


def _host_unlayout(y_core):
    """[P, ROWS*C] bf16 -> [ROWS, L] fp32."""
    return (
        y_core.reshape(P, ROWS, C)
        .transpose(1, 2, 0)
        .reshape(ROWS, L)
        .astype(np.float32)
    )


def _build():
    global _built
    if _built is not None:
        return _built

    from contextlib import ExitStack

    import concourse.bacc as bacc
    import concourse.mybir as mybir
    from concourse import tile

    f32 = mybir.dt.float32
    bf16 = mybir.dt.float16

    nc = bacc.Bacc("TRN2", target_bir_lowering=False, debug=False)

    XZ = nc.dram_tensor("xz", [P, ROWS * CP], bf16, kind="ExternalInput").ap()
    W = nc.dram_tensor("w", [P, 2 * P], bf16, kind="ExternalInput").ap()
    Y = nc.dram_tensor("y", [P, ROWS * C], bf16, kind="ExternalOutput").ap()

    with tile.TileContext(nc) as tc, ExitStack() as ctx:
        const_pool = ctx.enter_context(tc.tile_pool(name="const", bufs=1))
        x_pool = ctx.enter_context(tc.tile_pool(name="x", bufs=1))
        y_pool = ctx.enter_context(tc.tile_pool(name="y", bufs=1))
        po_pool = ctx.enter_context(tc.tile_pool(name="po", bufs=2, space="PSUM"))

        w_sb = const_pool.tile([P, 2 * P], bf16)
        nc.sync.dma_start(w_sb[:], W[:])

        xz_sb = x_pool.tile([P, ROWS * CP], bf16)
        y_sb = y_pool.tile([P, ROWS * C], bf16)

        # input DMAs: row 0 in halves (earlier compute start), rest whole-row
        nc.sync.dma_start(xz_sb[:, 0 : 2 * BANK + 1], XZ[:, 0 : 2 * BANK + 1])
        nc.sync.dma_start(xz_sb[:, 2 * BANK + 1 : CP], XZ[:, 2 * BANK + 1 : CP])
        for r in range(1, ROWS):
            nc.sync.dma_start(
                xz_sb[:, r * CP : (r + 1) * CP], XZ[:, r * CP : (r + 1) * CP]
            )

        for pr in range(ROWS // 2):  # row pairs share W0/W1 phases
            rows = (2 * pr, 2 * pr + 1)
            po = {r: po_pool.tile([P, C], f32, name=f"po{r}", tag="po") for r in rows}
            # W0 phase: one stationary load, 8 moving streams
            for r in rows:
                for b in range(NB):
                    nc.tensor.matmul(
                        po[r][:, b * BANK : (b + 1) * BANK],
                        w_sb[:, 0:P],
                        xz_sb[:, r * CP + 1 + b * BANK : r * CP + 1 + (b + 1) * BANK],
                        start=True,
                        stop=False,
                    )
            # W1 phase: spill from previous column (zero col handles t<0)
            for r in rows:
                for b in range(NB):
                    nc.tensor.matmul(
                        po[r][:, b * BANK : (b + 1) * BANK],
                        w_sb[:, P : 2 * P],
                        xz_sb[:, r * CP + b * BANK : r * CP + (b + 1) * BANK],
                        start=False,
                        stop=True,
                    )
            # evacuate PSUM -> bf16 SBUF (DVE and ACT split the banks), store
            for r in rows:
                for b in range(NB):
                    src = po[r][:, b * BANK : (b + 1) * BANK]
                    dst = y_sb[:, r * C + b * BANK : r * C + (b + 1) * BANK]
                    if b % 2 == 0:
                        nc.vector.tensor_copy(dst, src)
                    else:
                        nc.scalar.copy(dst, src)
                nc.scalar.dma_start(
                    Y[:, r * C : (r + 1) * C], y_sb[:, r * C : (r + 1) * C]
                )

    nc.compile()
    _built = nc
    return nc


def kernel(x, g, R, m_hp, m_bp, m_lp):
    x = np.ascontiguousarray(np.asarray(x, dtype=np.float32))
    h = _filter_taps(
        np.asarray(g).reshape(-1)[0],
        np.asarray(R).reshape(-1)[0],
        float(np.asarray(m_hp).reshape(-1)[0]),
        float(np.asarray(m_bp).reshape(-1)[0]),
        float(np.asarray(m_lp).reshape(-1)[0]),
    )
    w = _weights(h)

    nc = _build()
    from concourse.bass_utils import run_bass_kernel_spmd

    in_maps = [
        {"xz": _host_layout(x[c * ROWS : (c + 1) * ROWS]), "w": w}
        for c in range(N_CORES)
    ]
    global LAST_RESULTS
    kwargs = {}
    if TRACE:
        kwargs = {"trace": True, "tmpdir": TRACE_DIR}
    res = run_bass_kernel_spmd(nc, in_maps, list(range(N_CORES)), **kwargs)
    LAST_RESULTS = res
    y = np.concatenate(
        [_host_unlayout(res.results[c]["y"]) for c in range(N_CORES)], axis=0
    )
    return y.astype(np.float32, copy=False)


# revision 6
# speedup vs baseline: 1.8088x; 1.1128x over previous
"""Trainium2 Bass kernel for nn_DSVF (differentiable SVF filter, forward).

The reference applies an SVF biquad via FFT overlap-add (rfft/irfft at
NFFT=4096 over 2048-sample segments).  Because the biquad's poles are
well damped (radius ~0.5 for any plausible parameter draw), the aliased
impulse response decays below 1e-40 within 128 taps, so the whole
operation is numerically identical to a plain causal FIR applied to
each batch row (zero initial condition).  This kernel covers taps
0..255 exactly (truncation error ~|pole|^256, i.e. zero in fp32).

Layout (chosen so ALL device work is dense bf16 matmul + copies):
data-parallel over batch rows, 8 rows per core.  Each 262144-sample row
is framed column-major: z[k, c] = x[c*128 + k] (host-side transpose),
so SBUF holds [128 fine-time partitions x 2048+1 cols] per row with one
zero column prepended (zero initial condition / "col -1").

The FIR then splits into exactly two dense matmuls per output bank:
  out[m, c] = sum_k W0[k, m] z[k, c]  +  sum_k W1[k, m] z[k, c-1]
with W0[k, m] = h[m-k] (taps 0..127, lower-triangular Toeplitz) and
W1[k, m] = h[128+m-k] (taps 1..255, full).  W0/W1 are the *stationary*
operands — loaded once per phase — and the signal streams through as
512-wide bf16 moving operands, accumulating in PSUM (fp32).  The spill
across columns is just "same stream shifted one column", so there is no
halo duplication and no on-device transpose.

bf16 on the wire halves HBM traffic (the roofline: ~4.46 MB in +
4.19 MB out per core at ~358 GB/s ≈ 24 us); fp32 accumulation in PSUM
plus fp64 host tap computation keeps rel-err ~1e-3 << 2e-2.
"""

import os
import sys

import numpy as np
import ml_dtypes
ml_dtypes.float16 = __import__("numpy").float16  # fp16: more mantissa than bf16, same wire cost

for _p in ("/opt/trn_rl_repo",):
    if _p not in sys.path:
        sys.path.insert(0, _p)

N_CORES = 8
BATCH = 64
L = 262144
ROWS = BATCH // N_CORES  # rows per core
P = 128  # partitions == fine-time frame == contraction dim
C = L // P  # 2048 columns per row
CP = C + 1  # +1 zero column at each row start
T = 256  # FIR taps covered exactly (0..255)
BANK = 512  # PSUM bank width in fp32 == matmul moving width
NB = C // BANK  # 4 banks per row

_built = None

# Profiling knobs (used by the local test harness, not by grading):
TRACE = False
TRACE_DIR = None
LAST_RESULTS = None


def _filter_taps(g, R, m_hp, m_bp, m_lp):
    """First T taps of the biquad impulse response, float64 recursion."""
    g = float(g)
    R = float(R)
    gt = np.tan(np.pi * (1.0 / (1.0 + np.exp(-g))) / 2.0)
    Rt = np.log1p(np.exp(R))
    g2 = gt * gt
    b = (
        g2 * m_lp + gt * m_bp + m_hp,
        2 * g2 * m_lp - 2 * m_hp,
        g2 * m_lp - gt * m_bp + m_hp,
    )
    a = (g2 + 2 * Rt * gt + 1, 2 * g2 - 2, g2 - 2 * Rt * gt + 1)
    h = np.zeros(T, dtype=np.float64)
    for n in range(T):
        acc = b[n] if n < 3 else 0.0
        if n >= 1:
            acc -= a[1] * h[n - 1]
        if n >= 2:
            acc -= a[2] * h[n - 2]
        h[n] = acc / a[0]
    return h


def _weights(h):
    """[P, 2P] bf16: cols [0,P) = W0 (taps m-k), cols [P,2P) = W1 (128+m-k)."""
    k = np.arange(P)[:, None]
    m = np.arange(P)[None, :]
    d0 = m - k
    w0 = np.where(d0 >= 0, h[np.clip(d0, 0, T - 1)], 0.0)
    w1 = h[128 + d0]  # 128+m-k in [1, 255] always
    return np.concatenate([w0, w1], axis=1).astype(ml_dtypes.float16)


def _host_layout(x_shard):
    """[ROWS, L] fp32 -> [P, ROWS*CP] bf16, col-major frames + zero col."""
    z = np.zeros((P, ROWS * CP), dtype=ml_dtypes.float16)
    zt = x_shard.reshape(ROWS, C, P).transpose(0, 2, 1)  # [r, k, c]
    for r in range(ROWS):
        z[:, r * CP + 1 : (r + 1) * CP] = zt[r].astype(ml_dtypes.float16)
    return z


def _host_unlayout(y_core):
    """[P, ROWS*C] bf16 -> [ROWS, L] fp32."""
    return (
        y_core.reshape(P, ROWS, C)
        .transpose(1, 2, 0)
        .reshape(ROWS, L)
        .astype(np.float32)
    )


def _build():
    global _built
    if _built is not None:
        return _built

    from contextlib import ExitStack

    import concourse.bacc as bacc
    import concourse.mybir as mybir
    from concourse import tile

    f32 = mybir.dt.float32
    bf16 = mybir.dt.float16

    nc = bacc.Bacc("TRN2", target_bir_lowering=False, debug=False)

    XZ = nc.dram_tensor("xz", [P, ROWS * CP], bf16, kind="ExternalInput").ap()
    W = nc.dram_tensor("w", [P, 2 * P], bf16, kind="ExternalInput").ap()
    Y = nc.dram_tensor("y", [P, ROWS * C], bf16, kind="ExternalOutput").ap()

    with tile.TileContext(nc) as tc, ExitStack() as ctx:
        const_pool = ctx.enter_context(tc.tile_pool(name="const", bufs=1))
        x_pool = ctx.enter_context(tc.tile_pool(name="x", bufs=1))
        y_pool = ctx.enter_context(tc.tile_pool(name="y", bufs=1))
        po_pool = ctx.enter_context(tc.tile_pool(name="po", bufs=2, space="PSUM"))

        w_sb = const_pool.tile([P, 2 * P], bf16)
        nc.sync.dma_start(w_sb[:], W[:])

        xz_sb = x_pool.tile([P, ROWS * CP], bf16)
        y_sb = y_pool.tile([P, ROWS * C], bf16)

        # PE warmup on the (tiny, early) weight tile: keeps the HAM busy
        # window filling while the first row streams in, so real matmuls
        # start at 2.4 GHz instead of 1.2.
        po_w = po_pool.tile([P, 2 * BANK], f32, name="powarm", tag="po")
        for i in range(6):
            nc.tensor.matmul(
                po_w[:, 0:256],
                w_sb[:, 0:P],
                w_sb[:, 0:256],
                start=(i == 0),
                stop=(i == 5),
            )

        # input DMAs: issue split across both HWDGE rings (sync/scalar) so
        # descriptor generation (~0.6us each) runs in parallel; row 0 in
        # halves so compute starts as early as possible.
        nc.sync.dma_start(xz_sb[:, 0 : 2 * BANK + 1], XZ[:, 0 : 2 * BANK + 1])
        nc.sync.dma_start(xz_sb[:, 2 * BANK + 1 : CP], XZ[:, 2 * BANK + 1 : CP])
        for r in range(1, ROWS):
            eng = nc.scalar if r % 2 else nc.sync
            eng.dma_start(
                xz_sb[:, r * CP : (r + 1) * CP], XZ[:, r * CP : (r + 1) * CP]
            )

        for r in range(ROWS):
            last = r == ROWS - 1
            for h in range(2):  # half-rows: 2-bank PSUM tiles, bufs=4
                po = po_pool.tile([P, 2 * BANK], f32, name=f"po{r}_{h}", tag="po")
                for b in range(2):  # W0: in-column taps 0..127
                    col = h * 2 * BANK + b * BANK
                    nc.tensor.matmul(
                        po[:, b * BANK : (b + 1) * BANK],
                        w_sb[:, 0:P],
                        xz_sb[:, r * CP + 1 + col : r * CP + 1 + col + BANK],
                        start=True,
                        stop=False,
                    )
                for b in range(2):  # W1: spill taps 1..255 from prev column
                    col = h * 2 * BANK + b * BANK
                    nc.tensor.matmul(
                        po[:, b * BANK : (b + 1) * BANK],
                        w_sb[:, P : 2 * P],
                        xz_sb[:, r * CP + col : r * CP + col + BANK],
                        start=False,
                        stop=True,
                    )
                # evacuate this half-row immediately (frees the PSUM slot
                # fast): DVE takes one bank, ACT the other, in parallel.
                base = r * C + h * 2 * BANK
                nc.vector.tensor_copy(y_sb[:, base : base + BANK], po[:, 0:BANK])
                nc.scalar.copy(
                    y_sb[:, base + BANK : base + 2 * BANK], po[:, BANK : 2 * BANK]
                )
                if last:  # fine-grained final stores to shorten the tail
                    eng = nc.sync if h == 0 else nc.scalar
                    eng.dma_start(
                        Y[:, base : base + 2 * BANK], y_sb[:, base : base + 2 * BANK]
                    )
            if not last:  # one store per row, alternating HWDGE rings
                eng = nc.sync if r % 2 == 0 else nc.scalar
                eng.dma_start(Y[:, r * C : (r + 1) * C], y_sb[:, r * C : (r + 1) * C])

    nc.compile()
    _built = nc
    return nc


def kernel(x, g, R, m_hp, m_bp, m_lp):
    x = np.ascontiguousarray(np.asarray(x, dtype=np.float32))
    h = _filter_taps(
        np.asarray(g).reshape(-1)[0],
        np.asarray(R).reshape(-1)[0],
        float(np.asarray(m_hp).reshape(-1)[0]),
        float(np.asarray(m_bp).reshape(-1)[0]),
        float(np.asarray(m_lp).reshape(-1)[0]),
    )
    w = _weights(h)

    nc = _build()
    from concourse.bass_utils import run_bass_kernel_spmd

    in_maps = [
        {"xz": _host_layout(x[c * ROWS : (c + 1) * ROWS]), "w": w}
        for c in range(N_CORES)
    ]
    global LAST_RESULTS
    kwargs = {}
    if TRACE:
        kwargs = {"trace": True, "tmpdir": TRACE_DIR}
    res = run_bass_kernel_spmd(nc, in_maps, list(range(N_CORES)), **kwargs)
    LAST_RESULTS = res
    y = np.concatenate(
        [_host_unlayout(res.results[c]["y"]) for c in range(N_CORES)], axis=0
    )
    return y.astype(np.float32, copy=False)


# revision 7
# speedup vs baseline: 1.9417x; 1.0735x over previous
"""Trainium2 Bass kernel for nn_DSVF (differentiable SVF filter, forward).

The reference applies an SVF biquad via FFT overlap-add (rfft/irfft at
NFFT=4096 over 2048-sample segments).  Because the biquad's poles are
well damped (radius ~0.5 for any plausible parameter draw), the aliased
impulse response decays below 1e-40 within 128 taps, so the whole
operation is numerically identical to a plain causal FIR applied to
each batch row (zero initial condition).  This kernel covers taps
0..255 exactly (truncation error ~|pole|^256, i.e. zero in fp32).

Layout (chosen so ALL device work is dense bf16 matmul + copies):
data-parallel over batch rows, 8 rows per core.  Each 262144-sample row
is framed column-major: z[k, c] = x[c*128 + k] (host-side transpose),
so SBUF holds [128 fine-time partitions x 2048+1 cols] per row with one
zero column prepended (zero initial condition / "col -1").

The FIR then splits into exactly two dense matmuls per output bank:
  out[m, c] = sum_k W0[k, m] z[k, c]  +  sum_k W1[k, m] z[k, c-1]
with W0[k, m] = h[m-k] (taps 0..127, lower-triangular Toeplitz) and
W1[k, m] = h[128+m-k] (taps 1..255, full).  W0/W1 are the *stationary*
operands — loaded once per phase — and the signal streams through as
512-wide bf16 moving operands, accumulating in PSUM (fp32).  The spill
across columns is just "same stream shifted one column", so there is no
halo duplication and no on-device transpose.

bf16 on the wire halves HBM traffic (the roofline: ~4.46 MB in +
4.19 MB out per core at ~358 GB/s ≈ 24 us); fp32 accumulation in PSUM
plus fp64 host tap computation keeps rel-err ~1e-3 << 2e-2.
"""

import os
import sys

import numpy as np
import ml_dtypes
ml_dtypes.float16 = __import__("numpy").float16  # fp16: more mantissa than bf16, same wire cost

for _p in ("/opt/trn_rl_repo",):
    if _p not in sys.path:
        sys.path.insert(0, _p)

N_CORES = 8
BATCH = 64
L = 262144
ROWS = BATCH // N_CORES  # rows per core
P = 128  # partitions == fine-time frame == contraction dim
C = L // P  # 2048 columns per row
CP = C + 1  # +1 zero column at each row start
T = 256  # FIR taps covered exactly (0..255)
BANK = 512  # PSUM bank width in fp32 == matmul moving width
NB = C // BANK  # 4 banks per row

_built = None

# Profiling knobs (used by the local test harness, not by grading):
TRACE = False
TRACE_DIR = None
LAST_RESULTS = None


def _filter_taps(g, R, m_hp, m_bp, m_lp):
    """First T taps of the biquad impulse response, float64 recursion."""
    g = float(g)
    R = float(R)
    gt = np.tan(np.pi * (1.0 / (1.0 + np.exp(-g))) / 2.0)
    Rt = np.log1p(np.exp(R))
    g2 = gt * gt
    b = (
        g2 * m_lp + gt * m_bp + m_hp,
        2 * g2 * m_lp - 2 * m_hp,
        g2 * m_lp - gt * m_bp + m_hp,
    )
    a = (g2 + 2 * Rt * gt + 1, 2 * g2 - 2, g2 - 2 * Rt * gt + 1)
    h = np.zeros(T, dtype=np.float64)
    for n in range(T):
        acc = b[n] if n < 3 else 0.0
        if n >= 1:
            acc -= a[1] * h[n - 1]
        if n >= 2:
            acc -= a[2] * h[n - 2]
        h[n] = acc / a[0]
    return h


def _weights(h):
    """[P, 2P] bf16: cols [0,P) = W0 (taps m-k), cols [P,2P) = W1 (128+m-k)."""
    k = np.arange(P)[:, None]
    m = np.arange(P)[None, :]
    d0 = m - k
    w0 = np.where(d0 >= 0, h[np.clip(d0, 0, T - 1)], 0.0)
    w1 = h[128 + d0]  # 128+m-k in [1, 255] always
    return np.concatenate([w0, w1], axis=1).astype(ml_dtypes.float16)


def _host_layout(x_shard):
    """[ROWS, L] fp32 -> [P, ROWS*CP] bf16, col-major frames + zero col."""
    z = np.zeros((P, ROWS * CP), dtype=ml_dtypes.float16)
    zt = x_shard.reshape(ROWS, C, P).transpose(0, 2, 1)  # [r, k, c]
    for r in range(ROWS):
        z[:, r * CP + 1 : (r + 1) * CP] = zt[r].astype(ml_dtypes.float16)
    return z


def _host_unlayout(y_core):
    """[P, ROWS*C] bf16 -> [ROWS, L] fp32."""
    return (
        y_core.reshape(P, ROWS, C)
        .transpose(1, 2, 0)
        .reshape(ROWS, L)
        .astype(np.float32)
    )


def _build():
    global _built
    if _built is not None:
        return _built

    from contextlib import ExitStack

    import concourse.bacc as bacc
    import concourse.mybir as mybir
    from concourse import tile

    f32 = mybir.dt.float32
    bf16 = mybir.dt.float16

    nc = bacc.Bacc("TRN2", target_bir_lowering=False, debug=False)

    XZ = nc.dram_tensor("xz", [P, ROWS * CP], bf16, kind="ExternalInput").ap()
    W = nc.dram_tensor("w", [P, 2 * P], bf16, kind="ExternalInput").ap()
    Y = nc.dram_tensor("y", [P, ROWS * C], bf16, kind="ExternalOutput").ap()

    with tile.TileContext(nc) as tc, ExitStack() as ctx:
        const_pool = ctx.enter_context(tc.tile_pool(name="const", bufs=1))
        x_pool = ctx.enter_context(tc.tile_pool(name="x", bufs=1))
        y_pool = ctx.enter_context(tc.tile_pool(name="y", bufs=1))
        po_pool = ctx.enter_context(tc.tile_pool(name="po", bufs=2, space="PSUM"))

        w_sb = const_pool.tile([P, 2 * P], bf16)

        xz_sb = x_pool.tile([P, ROWS * CP], bf16)
        y_sb = y_pool.tile([P, ROWS * C], bf16)

        # input DMAs: issue split across both HWDGE rings (sync/scalar) so
        # descriptor generation (~0.6us each) runs in parallel and rows
        # arrive in consumption order; row 0 first (in halves) on sync, W
        # leads the scalar ring.
        nc.sync.dma_start(xz_sb[:, 0 : 2 * BANK + 1], XZ[:, 0 : 2 * BANK + 1])
        nc.scalar.dma_start(w_sb[:], W[:])
        nc.sync.dma_start(xz_sb[:, 2 * BANK + 1 : CP], XZ[:, 2 * BANK + 1 : CP])
        for r in range(1, ROWS):
            eng = nc.scalar if r % 2 else nc.sync
            eng.dma_start(
                xz_sb[:, r * CP : (r + 1) * CP], XZ[:, r * CP : (r + 1) * CP]
            )

        # PE warmup on the (tiny, early) weight tile: ~2us of matmul busy
        # while row 0 streams in keeps the HAM activity window filling, so
        # real matmuls reach 2.4 GHz quickly.
        po_w = po_pool.tile([P, 2 * BANK], f32, name="powarm", tag="po")
        for i in range(10):
            nc.tensor.matmul(
                po_w[:, 0:256],
                w_sb[:, 0:P],
                w_sb[:, 0:256],
                start=(i == 0),
                stop=(i == 9),
            )

        for r in range(ROWS):
            last = r == ROWS - 1
            for h in range(2):  # half-rows: 2-bank PSUM tiles, bufs=4
                po = po_pool.tile([P, 2 * BANK], f32, name=f"po{r}_{h}", tag="po")
                for b in range(2):  # W0: in-column taps 0..127
                    col = h * 2 * BANK + b * BANK
                    nc.tensor.matmul(
                        po[:, b * BANK : (b + 1) * BANK],
                        w_sb[:, 0:P],
                        xz_sb[:, r * CP + 1 + col : r * CP + 1 + col + BANK],
                        start=True,
                        stop=False,
                    )
                for b in range(2):  # W1: spill taps 1..255 from prev column
                    col = h * 2 * BANK + b * BANK
                    nc.tensor.matmul(
                        po[:, b * BANK : (b + 1) * BANK],
                        w_sb[:, P : 2 * P],
                        xz_sb[:, r * CP + col : r * CP + col + BANK],
                        start=False,
                        stop=True,
                    )
                # evacuate this half-row immediately (frees the PSUM slot
                # fast): DVE takes one bank, ACT the other, in parallel.
                base = r * C + h * 2 * BANK
                nc.vector.tensor_copy(y_sb[:, base : base + BANK], po[:, 0:BANK])
                nc.scalar.copy(
                    y_sb[:, base + BANK : base + 2 * BANK], po[:, BANK : 2 * BANK]
                )
                if last:  # fine-grained final stores to shorten the tail
                    eng = nc.sync if h == 0 else nc.scalar
                    eng.dma_start(
                        Y[:, base : base + 2 * BANK], y_sb[:, base : base + 2 * BANK]
                    )
            if not last:  # one store per row, alternating HWDGE rings
                eng = nc.sync if r % 2 == 0 else nc.scalar
                eng.dma_start(Y[:, r * C : (r + 1) * C], y_sb[:, r * C : (r + 1) * C])

    nc.compile()
    _built = nc
    return nc


def kernel(x, g, R, m_hp, m_bp, m_lp):
    x = np.ascontiguousarray(np.asarray(x, dtype=np.float32))
    h = _filter_taps(
        np.asarray(g).reshape(-1)[0],
        np.asarray(R).reshape(-1)[0],
        float(np.asarray(m_hp).reshape(-1)[0]),
        float(np.asarray(m_bp).reshape(-1)[0]),
        float(np.asarray(m_lp).reshape(-1)[0]),
    )
    w = _weights(h)

    nc = _build()
    from concourse.bass_utils import run_bass_kernel_spmd

    in_maps = [
        {"xz": _host_layout(x[c * ROWS : (c + 1) * ROWS]), "w": w}
        for c in range(N_CORES)
    ]
    global LAST_RESULTS
    kwargs = {}
    if TRACE:
        kwargs = {"trace": True, "tmpdir": TRACE_DIR}
    res = run_bass_kernel_spmd(nc, in_maps, list(range(N_CORES)), **kwargs)
    LAST_RESULTS = res
    y = np.concatenate(
        [_host_unlayout(res.results[c]["y"]) for c in range(N_CORES)], axis=0
    )
    return y.astype(np.float32, copy=False)


# revision 9
# speedup vs baseline: 1.9866x; 1.0231x over previous
"""Trainium2 Bass kernel for nn_DSVF (differentiable SVF filter, forward).

The reference applies an SVF biquad via FFT overlap-add (rfft/irfft at
NFFT=4096 over 2048-sample segments).  Because the biquad's poles are
well damped (radius ~0.5 for any plausible parameter draw), the aliased
impulse response decays below 1e-40 within 128 taps, so the whole
operation is numerically identical to a plain causal FIR applied to
each batch row (zero initial condition).  This kernel covers taps
0..255 exactly (truncation error ~|pole|^256, i.e. zero in fp32).

Layout (chosen so ALL device work is dense bf16 matmul + copies):
data-parallel over batch rows, 8 rows per core.  Each 262144-sample row
is framed column-major: z[k, c] = x[c*128 + k] (host-side transpose),
so SBUF holds [128 fine-time partitions x 2048+1 cols] per row with one
zero column prepended (zero initial condition / "col -1").

The FIR then splits into exactly two dense matmuls per output bank:
  out[m, c] = sum_k W0[k, m] z[k, c]  +  sum_k W1[k, m] z[k, c-1]
with W0[k, m] = h[m-k] (taps 0..127, lower-triangular Toeplitz) and
W1[k, m] = h[128+m-k] (taps 1..255, full).  W0/W1 are the *stationary*
operands — loaded once per phase — and the signal streams through as
512-wide bf16 moving operands, accumulating in PSUM (fp32).  The spill
across columns is just "same stream shifted one column", so there is no
halo duplication and no on-device transpose.

bf16 on the wire halves HBM traffic (the roofline: ~4.46 MB in +
4.19 MB out per core at ~358 GB/s ≈ 24 us); fp32 accumulation in PSUM
plus fp64 host tap computation keeps rel-err ~1e-3 << 2e-2.
"""

import os
import sys

import numpy as np
import ml_dtypes
ml_dtypes.float16 = __import__("numpy").float16  # fp16: more mantissa than bf16, same wire cost

for _p in ("/opt/trn_rl_repo",):
    if _p not in sys.path:
        sys.path.insert(0, _p)

N_CORES = 8
BATCH = 64
L = 262144
ROWS = BATCH // N_CORES  # rows per core
P = 128  # partitions == fine-time frame == contraction dim
C = L // P  # 2048 columns per row
CP = C + 1  # +1 zero column at each row start
T = 256  # FIR taps covered exactly (0..255)
BANK = 512  # PSUM bank width in fp32 == matmul moving width
NB = C // BANK  # 4 banks per row

_built = None

# Profiling knobs (used by the local test harness, not by grading):
TRACE = False
TRACE_DIR = None
LAST_RESULTS = None


def _filter_taps(g, R, m_hp, m_bp, m_lp):
    """First T taps of the biquad impulse response, float64 recursion."""
    g = float(g)
    R = float(R)
    gt = np.tan(np.pi * (1.0 / (1.0 + np.exp(-g))) / 2.0)
    Rt = np.log1p(np.exp(R))
    g2 = gt * gt
    b = (
        g2 * m_lp + gt * m_bp + m_hp,
        2 * g2 * m_lp - 2 * m_hp,
        g2 * m_lp - gt * m_bp + m_hp,
    )
    a = (g2 + 2 * Rt * gt + 1, 2 * g2 - 2, g2 - 2 * Rt * gt + 1)
    h = np.zeros(T, dtype=np.float64)
    for n in range(T):
        acc = b[n] if n < 3 else 0.0
        if n >= 1:
            acc -= a[1] * h[n - 1]
        if n >= 2:
            acc -= a[2] * h[n - 2]
        h[n] = acc / a[0]
    return h


def _weights(h):
    """[P, 2P] bf16: cols [0,P) = W0 (taps m-k), cols [P,2P) = W1 (128+m-k)."""
    k = np.arange(P)[:, None]
    m = np.arange(P)[None, :]
    d0 = m - k
    w0 = np.where(d0 >= 0, h[np.clip(d0, 0, T - 1)], 0.0)
    w1 = h[128 + d0]  # 128+m-k in [1, 255] always
    return np.concatenate([w0, w1], axis=1).astype(ml_dtypes.float16)


def _host_layout(x_shard):
    """[ROWS, L] fp32 -> [P, ROWS*CP] bf16, col-major frames + zero col."""
    z = np.zeros((P, ROWS * CP), dtype=ml_dtypes.float16)
    zt = x_shard.reshape(ROWS, C, P).transpose(0, 2, 1)  # [r, k, c]
    for r in range(ROWS):
        z[:, r * CP + 1 : (r + 1) * CP] = zt[r].astype(ml_dtypes.float16)
    return z


def _host_unlayout(y_core):
    """[P, ROWS*C] bf16 -> [ROWS, L] fp32."""
    return (
        y_core.reshape(P, ROWS, C)
        .transpose(1, 2, 0)
        .reshape(ROWS, L)
        .astype(np.float32)
    )


def _build():
    global _built
    if _built is not None:
        return _built

    from contextlib import ExitStack

    import concourse.bacc as bacc
    import concourse.mybir as mybir
    from concourse import tile

    f32 = mybir.dt.float32
    bf16 = mybir.dt.float16

    nc = bacc.Bacc("TRN2", target_bir_lowering=False, debug=False)

    XZ = nc.dram_tensor("xz", [P, ROWS * CP], bf16, kind="ExternalInput").ap()
    W = nc.dram_tensor("w", [P, 2 * P], bf16, kind="ExternalInput").ap()
    Y = nc.dram_tensor("y", [P, ROWS * C], bf16, kind="ExternalOutput").ap()

    with tile.TileContext(nc) as tc, ExitStack() as ctx:
        const_pool = ctx.enter_context(tc.tile_pool(name="const", bufs=1))
        x_pool = ctx.enter_context(tc.tile_pool(name="x", bufs=1))
        y_pool = ctx.enter_context(tc.tile_pool(name="y", bufs=1))
        po_pool = ctx.enter_context(tc.tile_pool(name="po", bufs=2, space="PSUM"))

        w_sb = const_pool.tile([P, 2 * P], bf16)

        xz_sb = x_pool.tile([P, ROWS * CP], bf16)
        y_sb = y_pool.tile([P, ROWS * C], bf16)

        # input DMAs: issue split across both HWDGE rings (sync/scalar) so
        # descriptor generation (~0.6us each) runs in parallel and rows
        # arrive in consumption order (even rows on the scalar ring, odd on
        # sync).  ALL output DMAs go on the sync ring, *behind* its input
        # rows in the queue FIFO: inputs drain at full rate first (they
        # gate the PE), and the output backlog then drains at full rate
        # instead of trickling at evac pace.
        nc.scalar.dma_start(xz_sb[:, 0 : 2 * BANK + 1], XZ[:, 0 : 2 * BANK + 1])
        nc.sync.dma_start(w_sb[:], W[:])
        nc.scalar.dma_start(xz_sb[:, 2 * BANK + 1 : CP], XZ[:, 2 * BANK + 1 : CP])
        for r in range(1, ROWS):
            eng = nc.sync if r % 2 else nc.scalar
            eng.dma_start(
                xz_sb[:, r * CP : (r + 1) * CP], XZ[:, r * CP : (r + 1) * CP]
            )

        # PE warmup on the (tiny, early) weight tile: ~2us of matmul busy
        # while row 0 streams in keeps the HAM activity window filling, so
        # real matmuls reach 2.4 GHz quickly.
        po_w = po_pool.tile([P, 2 * BANK], f32, name="powarm", tag="po")
        for i in range(10):
            nc.tensor.matmul(
                po_w[:, 0:256],
                w_sb[:, 0:P],
                w_sb[:, 0:256],
                start=(i == 0),
                stop=(i == 9),
            )

        for r in range(ROWS):
            last = r == ROWS - 1
            for h in range(2):  # half-rows: 2-bank PSUM tiles, bufs=4
                po = po_pool.tile([P, 2 * BANK], f32, name=f"po{r}_{h}", tag="po")
                for b in range(2):  # W0: in-column taps 0..127
                    col = h * 2 * BANK + b * BANK
                    nc.tensor.matmul(
                        po[:, b * BANK : (b + 1) * BANK],
                        w_sb[:, 0:P],
                        xz_sb[:, r * CP + 1 + col : r * CP + 1 + col + BANK],
                        start=True,
                        stop=False,
                    )
                for b in range(2):  # W1: spill taps 1..255 from prev column
                    col = h * 2 * BANK + b * BANK
                    nc.tensor.matmul(
                        po[:, b * BANK : (b + 1) * BANK],
                        w_sb[:, P : 2 * P],
                        xz_sb[:, r * CP + col : r * CP + col + BANK],
                        start=False,
                        stop=True,
                    )
                # evacuate this half-row immediately (frees the PSUM slot
                # fast): DVE takes one bank, ACT the other, in parallel.
                base = r * C + h * 2 * BANK
                nc.vector.tensor_copy(y_sb[:, base : base + BANK], po[:, 0:BANK])
                nc.scalar.copy(
                    y_sb[:, base + BANK : base + 2 * BANK], po[:, BANK : 2 * BANK]
                )
                if last:  # fine-grained final stores to shorten the tail
                    nc.sync.dma_start(
                        Y[:, base : base + 2 * BANK], y_sb[:, base : base + 2 * BANK]
                    )
            if not last:  # one store per row, behind the inputs in sync's FIFO
                nc.sync.dma_start(
                    Y[:, r * C : (r + 1) * C], y_sb[:, r * C : (r + 1) * C]
                )

    nc.compile()
    _built = nc
    return nc


def kernel(x, g, R, m_hp, m_bp, m_lp):
    x = np.ascontiguousarray(np.asarray(x, dtype=np.float32))
    h = _filter_taps(
        np.asarray(g).reshape(-1)[0],
        np.asarray(R).reshape(-1)[0],
        float(np.asarray(m_hp).reshape(-1)[0]),
        float(np.asarray(m_bp).reshape(-1)[0]),
        float(np.asarray(m_lp).reshape(-1)[0]),
    )
    w = _weights(h)

    nc = _build()
    from concourse.bass_utils import run_bass_kernel_spmd

    in_maps = [
        {"xz": _host_layout(x[c * ROWS : (c + 1) * ROWS]), "w": w}
        for c in range(N_CORES)
    ]
    global LAST_RESULTS
    kwargs = {}
    if TRACE:
        kwargs = {"trace": True, "tmpdir": TRACE_DIR}
    res = run_bass_kernel_spmd(nc, in_maps, list(range(N_CORES)), **kwargs)
    LAST_RESULTS = res
    y = np.concatenate(
        [_host_unlayout(res.results[c]["y"]) for c in range(N_CORES)], axis=0
    )
    return y.astype(np.float32, copy=False)


# revision 10
# speedup vs baseline: 2.1773x; 1.0960x over previous
"""Trainium2 Bass kernel for nn_DSVF (differentiable SVF filter, forward).

The reference applies an SVF biquad via FFT overlap-add (rfft/irfft at
NFFT=4096 over 2048-sample segments).  Because the biquad's poles are
well damped (radius ~0.5 for any plausible parameter draw), the aliased
impulse response decays below 1e-40 within 128 taps, so the whole
operation is numerically identical to a plain causal FIR applied to
each batch row (zero initial condition).  This kernel covers taps
0..255 exactly (truncation error ~|pole|^256, i.e. zero in fp32).

Layout (chosen so ALL device work is dense bf16 matmul + copies):
data-parallel over batch rows, 8 rows per core.  Each 262144-sample row
is framed column-major: z[k, c] = x[c*128 + k] (host-side transpose),
so SBUF holds [128 fine-time partitions x 2048+1 cols] per row with one
zero column prepended (zero initial condition / "col -1").

The FIR then splits into exactly two dense matmuls per output bank:
  out[m, c] = sum_k W0[k, m] z[k, c]  +  sum_k W1[k, m] z[k, c-1]
with W0[k, m] = h[m-k] (taps 0..127, lower-triangular Toeplitz) and
W1[k, m] = h[128+m-k] (taps 1..255, full).  W0/W1 are the *stationary*
operands — loaded once per phase — and the signal streams through as
512-wide bf16 moving operands, accumulating in PSUM (fp32).  The spill
across columns is just "same stream shifted one column", so there is no
halo duplication and no on-device transpose.

bf16 on the wire halves HBM traffic (the roofline: ~4.46 MB in +
4.19 MB out per core at ~358 GB/s ≈ 24 us); fp32 accumulation in PSUM
plus fp64 host tap computation keeps rel-err ~1e-3 << 2e-2.
"""

import os
import sys

import numpy as np
import ml_dtypes
ml_dtypes.float16 = __import__("numpy").float16  # fp16: more mantissa than bf16, same wire cost

for _p in ("/opt/trn_rl_repo",):
    if _p not in sys.path:
        sys.path.insert(0, _p)

N_CORES = 8
BATCH = 64
L = 262144
ROWS = BATCH // N_CORES  # rows per core
P = 128  # partitions == fine-time frame == contraction dim
C = L // P  # 2048 columns per row
CP = C + 1  # +1 zero column at each row start
T = 256  # FIR taps covered exactly (0..255)
BANK = 512  # PSUM bank width in fp32 == matmul moving width
NB = C // BANK  # 4 banks per row

_built = None

# Profiling knobs (used by the local test harness, not by grading):
TRACE = False
TRACE_DIR = None
LAST_RESULTS = None


def _filter_taps(g, R, m_hp, m_bp, m_lp):
    """First T taps of the biquad impulse response, float64 recursion."""
    g = float(g)
    R = float(R)
    gt = np.tan(np.pi * (1.0 / (1.0 + np.exp(-g))) / 2.0)
    Rt = np.log1p(np.exp(R))
    g2 = gt * gt
    b = (
        g2 * m_lp + gt * m_bp + m_hp,
        2 * g2 * m_lp - 2 * m_hp,
        g2 * m_lp - gt * m_bp + m_hp,
    )
    a = (g2 + 2 * Rt * gt + 1, 2 * g2 - 2, g2 - 2 * Rt * gt + 1)
    h = np.zeros(T, dtype=np.float64)
    for n in range(T):
        acc = b[n] if n < 3 else 0.0
        if n >= 1:
            acc -= a[1] * h[n - 1]
        if n >= 2:
            acc -= a[2] * h[n - 2]
        h[n] = acc / a[0]
    return h


def _weights(h):
    """[P, 2P] bf16: cols [0,P) = W0 (taps m-k), cols [P,2P) = W1 (128+m-k)."""
    k = np.arange(P)[:, None]
    m = np.arange(P)[None, :]
    d0 = m - k
    w0 = np.where(d0 >= 0, h[np.clip(d0, 0, T - 1)], 0.0)
    w1 = h[128 + d0]  # 128+m-k in [1, 255] always
    return np.concatenate([w0, w1], axis=1).astype(ml_dtypes.float16)


def _host_layout(x_shard):
    """[ROWS, L] fp32 -> [P, ROWS*CP] bf16, col-major frames + zero col."""
    z = np.zeros((P, ROWS * CP), dtype=ml_dtypes.float16)
    zt = x_shard.reshape(ROWS, C, P).transpose(0, 2, 1)  # [r, k, c]
    for r in range(ROWS):
        z[:, r * CP + 1 : (r + 1) * CP] = zt[r].astype(ml_dtypes.float16)
    return z


def _host_unlayout(y_core):
    """[P, ROWS*C] bf16 -> [ROWS, L] fp32."""
    return (
        y_core.reshape(P, ROWS, C)
        .transpose(1, 2, 0)
        .reshape(ROWS, L)
        .astype(np.float32)
    )


def _build():
    global _built
    if _built is not None:
        return _built

    from contextlib import ExitStack

    import concourse.bacc as bacc
    import concourse.mybir as mybir
    from concourse import tile

    f32 = mybir.dt.float32
    bf16 = mybir.dt.float16

    nc = bacc.Bacc("TRN2", target_bir_lowering=False, debug=False)

    XZ = nc.dram_tensor("xz", [P, ROWS * CP], bf16, kind="ExternalInput").ap()
    W = nc.dram_tensor("w", [P, 2 * P], bf16, kind="ExternalInput").ap()
    Y = nc.dram_tensor("y", [P, ROWS * C], bf16, kind="ExternalOutput").ap()

    with tile.TileContext(nc) as tc, ExitStack() as ctx:
        const_pool = ctx.enter_context(tc.tile_pool(name="const", bufs=1))
        x_pool = ctx.enter_context(tc.tile_pool(name="x", bufs=1))
        y_pool = ctx.enter_context(tc.tile_pool(name="y", bufs=1))
        po_pool = ctx.enter_context(tc.tile_pool(name="po", bufs=4, space="PSUM"))

        w_sb = const_pool.tile([P, 2 * P], bf16)

        xz_sb = x_pool.tile([P, ROWS * CP], bf16)
        y_sb = y_pool.tile([P, ROWS * C], bf16)

        # input DMAs: issue split across both HWDGE rings (sync/scalar) so
        # descriptor generation (~0.6us each) runs in parallel and rows
        # arrive in consumption order (even rows on the scalar ring, odd on
        # sync).  ALL output DMAs go on the sync ring, *behind* its input
        # rows in the queue FIFO: inputs drain at full rate first (they
        # gate the PE), and the output backlog then drains at full rate
        # instead of trickling at evac pace.
        nc.scalar.dma_start(xz_sb[:, 0 : 2 * BANK + 1], XZ[:, 0 : 2 * BANK + 1])
        nc.sync.dma_start(w_sb[:], W[:])
        nc.scalar.dma_start(xz_sb[:, 2 * BANK + 1 : CP], XZ[:, 2 * BANK + 1 : CP])
        for r in range(1, ROWS):
            eng = nc.sync if r % 2 else nc.scalar
            eng.dma_start(
                xz_sb[:, r * CP : (r + 1) * CP], XZ[:, r * CP : (r + 1) * CP]
            )

        # PE warmup on the (tiny, early) weight tile: ~2us of matmul busy
        # while row 0 streams in keeps the HAM activity window filling, so
        # real matmuls reach 2.4 GHz quickly.
        po_w = po_pool.tile([P, 2 * BANK], f32, name="powarm", tag="po")
        for i in range(10):
            nc.tensor.matmul(
                po_w[:, 0:256],
                w_sb[:, 0:P],
                w_sb[:, 0:256],
                start=(i == 0),
                stop=(i == 9),
            )

        for r in range(ROWS):
            last = r == ROWS - 1
            for h in range(2):  # half-rows: 2-bank PSUM tiles, bufs=4
                po = po_pool.tile([P, 2 * BANK], f32, name=f"po{r}_{h}", tag="po")
                for b in range(2):  # W0: in-column taps 0..127
                    col = h * 2 * BANK + b * BANK
                    nc.tensor.matmul(
                        po[:, b * BANK : (b + 1) * BANK],
                        w_sb[:, 0:P],
                        xz_sb[:, r * CP + 1 + col : r * CP + 1 + col + BANK],
                        start=True,
                        stop=False,
                    )
                for b in range(2):  # W1: spill taps 1..255 from prev column
                    col = h * 2 * BANK + b * BANK
                    nc.tensor.matmul(
                        po[:, b * BANK : (b + 1) * BANK],
                        w_sb[:, P : 2 * P],
                        xz_sb[:, r * CP + col : r * CP + col + BANK],
                        start=False,
                        stop=True,
                    )
                # evacuate this half-row immediately (frees the PSUM slot
                # fast): DVE takes one bank, ACT the other, in parallel.
                base = r * C + h * 2 * BANK
                nc.vector.tensor_copy(y_sb[:, base : base + BANK], po[:, 0:BANK])
                nc.scalar.copy(
                    y_sb[:, base + BANK : base + 2 * BANK], po[:, BANK : 2 * BANK]
                )
                if last:  # fine-grained final stores to shorten the tail
                    nc.sync.dma_start(
                        Y[:, base : base + 2 * BANK], y_sb[:, base : base + 2 * BANK]
                    )
            if not last:  # one store per row, behind the inputs in sync's FIFO
                nc.sync.dma_start(
                    Y[:, r * C : (r + 1) * C], y_sb[:, r * C : (r + 1) * C]
                )

    nc.compile()
    _built = nc
    return nc


def kernel(x, g, R, m_hp, m_bp, m_lp):
    x = np.ascontiguousarray(np.asarray(x, dtype=np.float32))
    h = _filter_taps(
        np.asarray(g).reshape(-1)[0],
        np.asarray(R).reshape(-1)[0],
        float(np.asarray(m_hp).reshape(-1)[0]),
        float(np.asarray(m_bp).reshape(-1)[0]),
        float(np.asarray(m_lp).reshape(-1)[0]),
    )
    w = _weights(h)

    nc = _build()
    from concourse.bass_utils import run_bass_kernel_spmd

    in_maps = [
        {"xz": _host_layout(x[c * ROWS : (c + 1) * ROWS]), "w": w}
        for c in range(N_CORES)
    ]
    global LAST_RESULTS
    kwargs = {}
    if TRACE:
        kwargs = {"trace": True, "tmpdir": TRACE_DIR}
    res = run_bass_kernel_spmd(nc, in_maps, list(range(N_CORES)), **kwargs)
    LAST_RESULTS = res
    y = np.concatenate(
        [_host_unlayout(res.results[c]["y"]) for c in range(N_CORES)], axis=0
    )
    return y.astype(np.float32, copy=False)


# revision 11
# speedup vs baseline: 2.1864x; 1.0042x over previous
"""Trainium2 Bass kernel for nn_DSVF (differentiable SVF filter, forward).

The reference applies an SVF biquad via FFT overlap-add (rfft/irfft at
NFFT=4096 over 2048-sample segments).  Because the biquad's poles are
well damped (radius ~0.5 for any plausible parameter draw), the aliased
impulse response decays below 1e-40 within 128 taps, so the whole
operation is numerically identical to a plain causal FIR applied to
each batch row (zero initial condition).  This kernel covers taps
0..255 exactly (truncation error ~|pole|^256, i.e. zero in fp32).

Layout (chosen so ALL device work is dense bf16 matmul + copies):
data-parallel over batch rows, 8 rows per core.  Each 262144-sample row
is framed column-major: z[k, c] = x[c*128 + k] (host-side transpose),
so SBUF holds [128 fine-time partitions x 2048+1 cols] per row with one
zero column prepended (zero initial condition / "col -1").

The FIR then splits into exactly two dense matmuls per output bank:
  out[m, c] = sum_k W0[k, m] z[k, c]  +  sum_k W1[k, m] z[k, c-1]
with W0[k, m] = h[m-k] (taps 0..127, lower-triangular Toeplitz) and
W1[k, m] = h[128+m-k] (taps 1..255, full).  W0/W1 are the *stationary*
operands — loaded once per phase — and the signal streams through as
512-wide bf16 moving operands, accumulating in PSUM (fp32).  The spill
across columns is just "same stream shifted one column", so there is no
halo duplication and no on-device transpose.

bf16 on the wire halves HBM traffic (the roofline: ~4.46 MB in +
4.19 MB out per core at ~358 GB/s ≈ 24 us); fp32 accumulation in PSUM
plus fp64 host tap computation keeps rel-err ~1e-3 << 2e-2.
"""

import os
import sys

import numpy as np
import ml_dtypes
ml_dtypes.float16 = __import__("numpy").float16  # fp16: more mantissa than bf16, same wire cost

for _p in ("/opt/trn_rl_repo",):
    if _p not in sys.path:
        sys.path.insert(0, _p)

N_CORES = 8
BATCH = 64
L = 262144
ROWS = BATCH // N_CORES  # rows per core
P = 128  # partitions == fine-time frame == contraction dim
C = L // P  # 2048 columns per row
CP = C + 1  # +1 zero column at each row start
T = 256  # FIR taps covered exactly (0..255)
BANK = 512  # PSUM bank width in fp32 == matmul moving width
NB = C // BANK  # 4 banks per row

_built = None

# Profiling knobs (used by the local test harness, not by grading):
TRACE = False
TRACE_DIR = None
LAST_RESULTS = None


def _filter_taps(g, R, m_hp, m_bp, m_lp):
    """First T taps of the biquad impulse response, float64 recursion."""
    g = float(g)
    R = float(R)
    gt = np.tan(np.pi * (1.0 / (1.0 + np.exp(-g))) / 2.0)
    Rt = np.log1p(np.exp(R))
    g2 = gt * gt
    b = (
        g2 * m_lp + gt * m_bp + m_hp,
        2 * g2 * m_lp - 2 * m_hp,
        g2 * m_lp - gt * m_bp + m_hp,
    )
    a = (g2 + 2 * Rt * gt + 1, 2 * g2 - 2, g2 - 2 * Rt * gt + 1)
    h = np.zeros(T, dtype=np.float64)
    for n in range(T):
        acc = b[n] if n < 3 else 0.0
        if n >= 1:
            acc -= a[1] * h[n - 1]
        if n >= 2:
            acc -= a[2] * h[n - 2]
        h[n] = acc / a[0]
    return h


def _weights(h):
    """[P, 2P] bf16: cols [0,P) = W0 (taps m-k), cols [P,2P) = W1 (128+m-k)."""
    k = np.arange(P)[:, None]
    m = np.arange(P)[None, :]
    d0 = m - k
    w0 = np.where(d0 >= 0, h[np.clip(d0, 0, T - 1)], 0.0)
    w1 = h[128 + d0]  # 128+m-k in [1, 255] always
    return np.concatenate([w0, w1], axis=1).astype(ml_dtypes.float16)


def _host_layout(x_shard):
    """[ROWS, L] fp32 -> [P, ROWS*CP] bf16, col-major frames + zero col."""
    z = np.zeros((P, ROWS * CP), dtype=ml_dtypes.float16)
    zt = x_shard.reshape(ROWS, C, P).transpose(0, 2, 1)  # [r, k, c]
    for r in range(ROWS):
        z[:, r * CP + 1 : (r + 1) * CP] = zt[r].astype(ml_dtypes.float16)
    return z


def _host_unlayout(y_core):
    """[P, ROWS*C] bf16 -> [ROWS, L] fp32."""
    return (
        y_core.reshape(P, ROWS, C)
        .transpose(1, 2, 0)
        .reshape(ROWS, L)
        .astype(np.float32)
    )


def _build():
    global _built
    if _built is not None:
        return _built

    from contextlib import ExitStack

    import concourse.bacc as bacc
    import concourse.mybir as mybir
    from concourse import tile

    f32 = mybir.dt.float32
    bf16 = mybir.dt.float16

    nc = bacc.Bacc("TRN2", target_bir_lowering=False, debug=False)

    XZ = nc.dram_tensor("xz", [P, ROWS * CP], bf16, kind="ExternalInput").ap()
    W = nc.dram_tensor("w", [P, 2 * P], bf16, kind="ExternalInput").ap()
    Y = nc.dram_tensor("y", [P, ROWS * C], bf16, kind="ExternalOutput").ap()

    with tile.TileContext(nc) as tc, ExitStack() as ctx:
        const_pool = ctx.enter_context(tc.tile_pool(name="const", bufs=1))
        x_pool = ctx.enter_context(tc.tile_pool(name="x", bufs=1))
        y_pool = ctx.enter_context(tc.tile_pool(name="y", bufs=1))
        po_pool = ctx.enter_context(tc.tile_pool(name="po", bufs=4, space="PSUM"))

        w_sb = const_pool.tile([P, 2 * P], bf16)

        xz_sb = x_pool.tile([P, ROWS * CP], bf16)
        y_sb = y_pool.tile([P, ROWS * C], bf16)

        # input DMAs: issue split across both HWDGE rings (sync/scalar) so
        # descriptor generation (~0.6us each) runs in parallel and rows
        # arrive in consumption order (even rows on the scalar ring, odd on
        # sync).  ALL output DMAs go on the sync ring, *behind* its input
        # rows in the queue FIFO: inputs drain at full rate first (they
        # gate the PE), and the output backlog then drains at full rate
        # instead of trickling at evac pace.
        nc.scalar.dma_start(xz_sb[:, 0 : 2 * BANK + 1], XZ[:, 0 : 2 * BANK + 1])
        nc.sync.dma_start(w_sb[:], W[:])
        nc.scalar.dma_start(xz_sb[:, 2 * BANK + 1 : CP], XZ[:, 2 * BANK + 1 : CP])
        nc.sync.dma_start(xz_sb[:, CP : 2 * CP], XZ[:, CP : 2 * CP])
        nc.scalar.dma_start(xz_sb[:, 2 * CP : 3 * CP], XZ[:, 2 * CP : 3 * CP])
        nc.sync.dma_start(xz_sb[:, 3 * CP : 4 * CP], XZ[:, 3 * CP : 4 * CP])
        nc.scalar.dma_start(xz_sb[:, 4 * CP : 6 * CP], XZ[:, 4 * CP : 6 * CP])
        nc.sync.dma_start(xz_sb[:, 6 * CP : 8 * CP], XZ[:, 6 * CP : 8 * CP])

        # PE warmup on the (tiny, early) weight tile: ~2us of matmul busy
        # while row 0 streams in keeps the HAM activity window filling, so
        # real matmuls reach 2.4 GHz quickly.
        po_w = po_pool.tile([P, 2 * BANK], f32, name="powarm", tag="po")
        for i in range(4):
            nc.tensor.matmul(
                po_w[:, 0:256],
                w_sb[:, 0:P],
                w_sb[:, 0:256],
                start=(i == 0),
                stop=(i == 3),
            )

        for r in range(ROWS):
            last = r == ROWS - 1
            for h in range(2):  # half-rows: 2-bank PSUM tiles, bufs=4
                po = po_pool.tile([P, 2 * BANK], f32, name=f"po{r}_{h}", tag="po")
                for b in range(2):  # W0: in-column taps 0..127
                    col = h * 2 * BANK + b * BANK
                    nc.tensor.matmul(
                        po[:, b * BANK : (b + 1) * BANK],
                        w_sb[:, 0:P],
                        xz_sb[:, r * CP + 1 + col : r * CP + 1 + col + BANK],
                        start=True,
                        stop=False,
                    )
                for b in range(2):  # W1: spill taps 1..255 from prev column
                    col = h * 2 * BANK + b * BANK
                    nc.tensor.matmul(
                        po[:, b * BANK : (b + 1) * BANK],
                        w_sb[:, P : 2 * P],
                        xz_sb[:, r * CP + col : r * CP + col + BANK],
                        start=False,
                        stop=True,
                    )
                # evacuate this half-row in one 1024-wide copy (fewer
                # instructions => fewer event sems to clear in the epilogue);
                # DVE and ACT alternate by half-row.
                base = r * C + h * 2 * BANK
                if h == 0:
                    nc.vector.tensor_copy(y_sb[:, base : base + 2 * BANK], po[:, 0 : 2 * BANK])
                else:
                    nc.scalar.copy(y_sb[:, base : base + 2 * BANK], po[:, 0 : 2 * BANK])
                if last:  # fine-grained final stores to shorten the tail
                    nc.sync.dma_start(
                        Y[:, base : base + 2 * BANK], y_sb[:, base : base + 2 * BANK]
                    )
            if r in (1, 3, 5):  # pair stores, behind the inputs in sync's FIFO
                nc.sync.dma_start(
                    Y[:, (r - 1) * C : (r + 1) * C], y_sb[:, (r - 1) * C : (r + 1) * C]
                )
            elif r == 6:
                nc.sync.dma_start(
                    Y[:, r * C : (r + 1) * C], y_sb[:, r * C : (r + 1) * C]
                )

    nc.compile()
    _built = nc
    return nc


def kernel(x, g, R, m_hp, m_bp, m_lp):
    x = np.ascontiguousarray(np.asarray(x, dtype=np.float32))
    h = _filter_taps(
        np.asarray(g).reshape(-1)[0],
        np.asarray(R).reshape(-1)[0],
        float(np.asarray(m_hp).reshape(-1)[0]),
        float(np.asarray(m_bp).reshape(-1)[0]),
        float(np.asarray(m_lp).reshape(-1)[0]),
    )
    w = _weights(h)

    nc = _build()
    from concourse.bass_utils import run_bass_kernel_spmd

    in_maps = [
        {"xz": _host_layout(x[c * ROWS : (c + 1) * ROWS]), "w": w}
        for c in range(N_CORES)
    ]
    global LAST_RESULTS
    kwargs = {}
    if TRACE:
        kwargs = {"trace": True, "tmpdir": TRACE_DIR}
    res = run_bass_kernel_spmd(nc, in_maps, list(range(N_CORES)), **kwargs)
    LAST_RESULTS = res
    y = np.concatenate(
        [_host_unlayout(res.results[c]["y"]) for c in range(N_CORES)], axis=0
    )
    return y.astype(np.float32, copy=False)
